# revision 12
# baseline (speedup 1.0000x reference)
"""Cost volume layer (search range 4, 81 channels) on 8 trn2 cores.

Per-core (batch-parallel): out[k,h,w] = (1/81) * sum_c x1[c,h,w] * x2p[c,h+ii,w+jj]
where x2p is x2 zero-padded by 4 on h/w, k = (40 - 9*ii - jj) mod 81, ii,jj in 0..8.

Pipeline per core:
 1. Load x2 row-strips (padded, zero halo) + x1 strips into SBUF (f32).
 2. PE: per (h, w-chunk of 128): 3 matmuls (fp32r, N=408) -> psum bands
    band[m, 512t+136tt+n] = sum_c x1[c,h,w0+m] * x2p[c,h+3t+tt,w0+n].
 3. ACT/DVE: compact-evict diagonal g-blocks [32, 40] of all 9 bands into
    S[m, n_local*9 + ii] (scale 1/81) -- keeps 360 of 1224 band cols.
 4. DMA S -> DRAM scratch (big rect runs).
 5. DMA scratch -> G[128, Hs*81] with diagonal HBM-side AP: per pixel m one
    contiguous 81-elem run holding all channels r = 9*jj + ii.
 6. PE transpose G[:, h'*81:+81] -> TP[81, 128] psum; evict to T[81, Hs*W] f32.
 7. 10 final DMAs per strip: T -> out[k(r), h, w] via negative-stride k mapping.
"""
import sys

sys.path.insert(0, "/opt/trn_rl_repo")

import numpy as np
import bass_rust

SR = 4
D = 9
NCH = 81
C = 64
H = 192
W = 256
Wp = W + 2 * SR          # 264 padded width
M = 128                  # pixels per w-chunk
NW = M + 2 * SR          # 136 window width per chunk
NCHUNK = W // M          # 2
GB = 32                  # pixels per g-block
NG = M // GB             # 4
GW = GB + 2 * SR         # 40 window per g-block
SWID = GW * D            # 360 S-width per pixel
PS_T = 512               # psum cols per i-triple slot
N_T = 3 * NW             # 408 matmul N


def _cap(base_ap, dims, extra_offset=0):
    """Custom access pattern: copy of base_ap with overridden dims/offset."""
    a = base_ap.copy()
    a.ap = bass_rust.VecI64Pair(dims)
    if extra_offset:
        a.offset = a.offset + extra_offset
    return a


def build_kernel(nc, Hs=16, use_fp32r=True, scratch_bf16=True, Hloc=H):
    import concourse.mybir as mybir
    import concourse.tile as tile
    from concourse.masks import make_identity

    f32 = mybir.dt.float32
    dt_s = mybir.dt.bfloat16 if scratch_bf16 else mybir.dt.float32
    dt_mm = mybir.dt.bfloat16 if use_fp32r else mybir.dt.float32
    conv = use_fp32r  # convert inputs to bf16 for 1-cycle matmuls

    assert Hloc % Hs == 0
    nstrip = Hloc // Hs

    x1 = nc.dram_tensor("x1", [C, Hloc, W], f32, kind="ExternalInput")
    x2 = nc.dram_tensor("x2", [C, Hloc, W], f32, kind="ExternalInput")
    out = nc.dram_tensor("out", [NCH, Hloc, W], f32, kind="ExternalOutput")
    HWloc = Hloc * W

    # final DMA pieces: (ii, jj0, njj, k_at_jj0); k(jj) = k_at_jj0 - (jj - jj0)
    pieces = []  # (iip, jj0, njj, kstart); k = kstart + (jj' - jj0), ascending
    for iip in range(D):
        k0 = 9 * iip - 40
        if iip <= 3:
            pieces.append((iip, 0, 9, k0 + 81))
        elif iip == 4:
            pieces.append((4, 0, 4, 77))
            pieces.append((4, 4, 5, 0))
        else:
            pieces.append((iip, 0, 9, k0))

    x2w = (Hs + 8) * Wp  # x2 strip tile free width
    sw_all = Hs * NCHUNK * SWID  # S staging free width

    with tile.TileContext(nc) as tc:
        with (
            tc.tile_pool(name="consts", bufs=1) as consts,
            tc.tile_pool(name="x1p", bufs=1 if conv else 2) as x1p,
            tc.tile_pool(name="x1bfp", bufs=2) as x1bfp,
            tc.tile_pool(name="x2bfp", bufs=2) as x2bfp,
            tc.tile_pool(name="x2pool", bufs=1 if conv else 2) as x2pool,
            tc.tile_pool(name="sp", bufs=2 if scratch_bf16 else 1) as sp,
            tc.tile_pool(name="gp", bufs=2) as gp,
            tc.tile_pool(name="tpool", bufs=2) as tpool,
            tc.tile_pool(name="psum", bufs=4, space="PSUM") as pspool,
            tc.tile_pool(name="psumT", bufs=2, space="PSUM") as pstpool,
            tc.tile_pool(name="scr", bufs=4, space="DRAM") as scrpool,
        ):
            ident = consts.tile([128, 128], dt_s)
            make_identity(nc, ident[:])

            for s in range(nstrip):
                hs = s * Hs
                # ---- x1 strip [64, Hs*W] f32
                x1t = x1p.tile([C, Hs * W], f32)
                nc.sync.dma_start(x1t[:], x1[:, hs : hs + Hs, :])
                # ---- x2 strip [64, (Hs+8)*Wp] f32, zero-padded halo
                x2t = x2pool.tile([C, x2w], f32)
                nc.vector.memset(
                    _cap(x2t[:], [[x2w, C], [Wp, Hs + 8], [1, SR]]), 0.0
                )
                nc.vector.memset(
                    _cap(x2t[:], [[x2w, C], [Wp, Hs + 8], [1, SR]], Wp - SR), 0.0
                )
                p_lo = max(hs, SR)           # first valid padded row
                p_hi = min(hs + Hs + 8, Hloc + SR)
                if p_lo > hs:
                    nc.vector.memset(
                        _cap(x2t[:], [[x2w, C], [Wp, p_lo - hs], [1, W]], SR), 0.0
                    )
                if p_hi < hs + Hs + 8:
                    nc.vector.memset(
                        _cap(
                            x2t[:],
                            [[x2w, C], [Wp, hs + Hs + 8 - p_hi], [1, W]],
                            (p_hi - hs) * Wp + SR,
                        ),
                        0.0,
                    )
                nrows = p_hi - p_lo
                nc.sync.dma_start(
                    _cap(x2t[:], [[x2w, C], [Wp, nrows], [1, W]], (p_lo - hs) * Wp + SR),
                    _cap(x2[:], [[HWloc, C], [W, nrows], [1, W]], (p_lo - SR) * W),
                )
                if conv:
                    x1mm = x1bfp.tile([C, Hs * W], dt_mm)
                    nc.vector.tensor_copy(x1mm[:], x1t[:])
                    x2mm = x2bfp.tile([C, x2w], dt_mm)
                    nc.scalar.copy(x2mm[:], x2t[:])
                else:
                    x1mm, x2mm = x1t, x2t

                scr = [
                    scrpool.tile([Hs * 128, SWID], dt_s, tag=f"scr{c_}", name=f"scr_s{s}c{c_}")
                    for c_ in range(NCHUNK)
                ]
                st = sp.tile([128, sw_all], dt_s)

                for hp in range(Hs):
                    for chunk in range(NCHUNK):
                        w0 = chunk * M
                        ps = pspool.tile([128, PS_T], f32)
                        for g in range(NG):
                            lhsT = _cap(
                                x1mm[:], [[Hs * W, C], [1, GB]], hp * W + w0 + g * GB
                            )
                            rhs = _cap(
                                x2mm[:],
                                [[x2w, C], [Wp, D], [1, GW]],
                                hp * Wp + w0 + g * GB,
                            )
                            nc.tensor.matmul(
                                ps[g * GB : (g + 1) * GB, 0:SWID],
                                lhsT,
                                rhs,
                                start=True,
                                stop=True,
                                tile_position=(0, g * GB),
                            )
                        # one full-lane reversed compact eviction per (h, chunk)
                        slot = (hp * NCHUNK + chunk) * SWID
                        pin = _cap(
                            ps[:],
                            [[PS_T, 128], [-GW, D], [-1, GW]],
                            (D - 1) * GW + (GW - 1),
                        )
                        pout = _cap(
                            st[:],
                            [[sw_all, 128], [1, D], [D, GW]],
                            slot,
                        )
                        if (hp + chunk) % 2 == 0:
                            nc.scalar.mul(pout, pin, 1.0 / NCH)
                        else:
                            nc.vector.tensor_scalar_mul(pout, pin, 1.0 / NCH)

                # S -> scratch, one DMA per chunk
                for chunk in range(NCHUNK):
                    src = _cap(
                        st[:],
                        [[sw_all, 128], [NCHUNK * SWID, Hs], [1, SWID]],
                        chunk * SWID,
                    )
                    dst = _cap(scr[chunk][:], [[SWID, 128], [128 * SWID, Hs], [1, SWID]])
                    nc.gpsimd.dma_start(dst, src)

                # T tile covers the whole strip (both chunks)
                tt_ = tpool.tile([NCH, Hs * W], f32)
                for chunk in range(NCHUNK):
                    w0 = chunk * M
                    gt = gp.tile([128, Hs * NCH], dt_s)
                    for g in range(NG):
                        src = _cap(
                            scr[chunk][:],
                            [[SWID - D, GB], [128 * SWID, Hs], [1, NCH]],
                            g * GB * SWID + (GW - 1 - 8) * D + 0 * 8 + 0,
                        )
                        dstg = _cap(
                            gt[:],
                            [[Hs * NCH, GB], [NCH, Hs], [1, NCH]],
                            g * GB * (Hs * NCH),
                        )
                        nc.gpsimd.dma_start(dstg, src)
                    for hp in range(Hs):
                        tp_ps = pstpool.tile([NCH, 128], dt_s)
                        nc.tensor.transpose(
                            tp_ps[:], gt[:, hp * NCH : (hp + 1) * NCH], ident[:]
                        )
                        dst_t = _cap(tt_[:], [[Hs * W, NCH], [1, M]], hp * W + w0)
                        if (hp + chunk) % 2 == 0:
                            nc.vector.tensor_copy(dst_t, tp_ps[:])
                        else:
                            nc.scalar.copy(dst_t, tp_ps[:])

                # final DMAs: 10 pieces
                for (iip, jj0, njj, kstart) in pieces:
                    src = _cap(
                        tt_[:],
                        [[D * (Hs * W), njj], [W, Hs], [1, W]],
                        (iip + D * jj0) * (Hs * W),
                    )
                    dst = _cap(
                        out[:],
                        [[HWloc, njj], [W, Hs], [1, W]],
                        kstart * HWloc + hs * W,
                    )
                    nc.sync.dma_start(dst, src)
    return nc


# ---------------- host side ----------------
_N_CORES = 8


def kernel(x1: np.ndarray, x2: np.ndarray) -> np.ndarray:
    from concourse import bacc
    from concourse.bass_utils import run_bass_kernel_spmd

    B = x1.shape[0]
    nc = bacc.Bacc("TRN2", target_bir_lowering=False, debug=False, num_devices=_N_CORES)
    build_kernel(nc)
    nc.compile()
    in_maps = [
        {"x1": np.ascontiguousarray(x1[b]), "x2": np.ascontiguousarray(x2[b])}
        for b in range(B)
    ]
    res = run_bass_kernel_spmd(nc, in_maps, core_ids=list(range(_N_CORES)))
    return np.stack([res.results[b]["out"] for b in range(B)], axis=0)
